# revision 39
# baseline (speedup 1.0000x reference)
"""AGD loss (angular-Gaussian density contrastive loss) on 8 TRN2 NeuronCores.

Math.  Per column j (n = V*B = 32768 view-major columns) and class c (C = 100)
the reference evaluates the 40-term Saw-series density s(y[c,j]),
    s(a) = sum_n c_n a^n,   c_n = 2^{n/2} Gamma((d+n)/2) / (Gamma(d/2) n!),
takes norms_j = sum_c s(y[c,j]) and the own-class s(y[label_j, j]), and sums
-(log s_lab - log norms).  The huge exp(log_Cd - 1/(2 sigma^2)) prefactor
cancels in the log-ratio, so the kernel works with s directly in 16/32-bit.

The key identity: c_n * n! are exactly the raw moments of a chi(d=128)
variable R, so s(a) = E_R[e^{R a}].  A 3-node equispaced-exponential fit
    s(a) ~= A * ((B' + W1P) * B' + W0),   B' = e^{DLT a + B0},  A = e^{R0 a}
(nodes R0, R0+DLT, R0+2DLT; leading weight folded into B0) reproduces s to
1.9e-5 max rel on |a| <= 0.6; the constants are then re-centered against the
exact fp16 rounding staircase of the device pipeline (bias+variance objective
on the enumerable fp16 grid, weighted by the unit-sphere coordinate density),
which drives the end-to-end loss error to ~2e-7.

Per core (data-parallel over columns, 4096 columns/core, shard = rows
0..99 = y classes, row 100 = host-gathered own-class value, 101..127 zero):
  - input fp16 [128, w] chunks (512/1536/1536/512 columns), SWDGE DMA
    (128 partitions => full 16-engine spray, ~330 GB/s)
  - ScalarE: 2 Exp passes per chunk (B', A) with fused scale+bias
  - VectorE: quadratic Horner in fp16: tensor_scalar add (4x mode) +
    tensor_tensor mult (2x mode), twice
  - TensorE: [128 -> 2] matmul per 512-column bank against a constant
    (ones | one-hot-row-100) selection matrix; banks of 3 land at PSUM
    partition offsets 0/32/64 of a shared tile
  - ScalarE: one Ln (FD=512) per 3-bank group with accum_out -> [66, 3]
    partial sums; per-group DMA out
  - host: loss = sum over (norm - lab) partial-sum pairs in float64
All activation biases are explicit SBUF tiles (no const-AP reads), which
allows skipping the init-time all-engine barrier; the Tile tail is trimmed
to the global drain (re-execution verified bit-identical).
"""

import numpy as np

import concourse.bass as bass
import concourse.bacc as bacc
import concourse.mybir as mybir
from concourse.tile import TileContext
from concourse.bass_utils import run_bass_kernel_spmd

N_CORES = 8
B = 16384
V = 2
D = 128
C = 100
N = V * B                 # 32768 columns
NLOC = N // N_CORES       # 4096 columns per core
P = 128                   # 100 class rows + 1 own-class row + 27 zero pad rows
MM_N = 512                # matmul moving free dim (one PSUM bank)
NGRP = 3                  # Ln groups: banks {0,1,2}, {3,4,5}, {6,7}

# Equispaced 3-node exponential fit of the chi(128) MGF on |a| <= 0.6, with
# the leading weight folded into the B exponent and all constants re-centered
# against the exact fp16 rounding staircase of this pipeline:
#   s(a) ~= A * ((B' + W1P) * B' + W0),  B' = e^{DLT a + B0},  A = e^{R0 a}
# (raw fit 1.9e-5 max rel; tuned end-to-end loss error ~6e-7)
R0 = 9.785
DLT = 1.3
B0 = -0.7141612172077164
W1P = 1.387914841360724
W0 = 0.08100894876678372

CHUNKS = [512, 1536, 1536, 512]   # columns per chunk (banks 1/3/3/1)

IN_DT = mybir.dt.float16

_CACHE = {}
LAST_RESULT = None  # BassKernelResults of the most recent run (for profiling)
TRACE = False


class _scoped_patches:
    """Scoped (build-time only) framework tweaks:
    - Tile end-of-kernel: keep only the global drain (it waits for all
      outstanding work incl. the output DMAs); skip the per-semaphore clear
      storm and the two all-engine barriers.  Re-execution stays correct
      (verified bit-identical across runs) since semaphore state is reset
      at NEFF (re)start.
    - Map both Exp and Ln onto the single natural_log_exp_and_others ACT
      table set (one ~2.7us table load instead of two).
    - Skip the Bass-init all-engine barrier; its only role here is ordering
      the const-AP memsets, which this kernel never reads (all activation
      biases are explicit tiles)."""

    def __enter__(self):
        from concourse import tile as tile_mod
        import concourse.hw_specs as hw_specs

        def drain_only(tc_self, tick_clock, wait_clock):
            # The drain must wait for DMA *completion* (HBM write receipts of
            # the output DMAs) — queue-empty alone is not enough.  But the
            # compute-engine/sequencer clock entries are redundant: every
            # engine's work is ordered before the output DMAs via their own
            # data-dependency waits.  Keep only the 16 DMA lane ticks.
            from concourse.tile_sem_assignment import N_PROCS

            gc = tick_clock.global_clock
            filt = tile_mod.VectorClock(
                [gc[p] if p >= 11 else 0 for p in range(N_PROCS)]
            )
            drain_inst = tc_self.nc.sync.drain()
            wait_clock.add_sem_waits(
                drain_inst.ins, tile_mod.ScopedClock({None: filt})
            )
            popped = tc_self.nc._tile_sem_poison_stack.pop()
            assert popped is tc_self._sem_poison

        orig_tables = hw_specs.get_activation_tables

        def patched_tables(module_arch):
            tabs = orig_tables(module_arch)
            exp_f = mybir.ActivationFunctionType.Exp
            ln_f = mybir.ActivationFunctionType.Ln
            out = {}
            for name, funcs in tabs.items():
                if name != "natural_log_exp_and_others" and (
                    exp_f in funcs or ln_f in funcs
                ):
                    funcs = funcs - {exp_f, ln_f}
                out[name] = funcs
            return out

        self._saved = (
            tile_mod.TileContext._drain_and_barrier,
            hw_specs.get_activation_tables,
            bacc.get_activation_tables,
            bass.Bass.all_engine_barrier,
        )
        self._mods = (tile_mod, hw_specs)
        tile_mod.TileContext._drain_and_barrier = drain_only
        hw_specs.get_activation_tables = patched_tables
        bacc.get_activation_tables = patched_tables
        bass.Bass.all_engine_barrier = lambda nc_self, **kw: None
        return self

    def __exit__(self, *exc):
        tile_mod, hw_specs = self._mods
        (
            tile_mod.TileContext._drain_and_barrier,
            hw_specs.get_activation_tables,
            bacc.get_activation_tables,
            bass.Bass.all_engine_barrier,
        ) = self._saved
        return False


def build_bass():
    with _scoped_patches():
        return _build_bass_inner()


def _build_bass_inner():
    nc = bacc.Bacc(None, target_bir_lowering=False)
    xs = [
        nc.declare_dram_parameter(f"x{k}", [P, w], IN_DT, isOutput=False)
        for k, w in enumerate(CHUNKS)
    ]
    sel_in = nc.declare_dram_parameter("sel", [P, 2], IN_DT, isOutput=False)
    out = nc.declare_dram_parameter("out", [66, NGRP], mybir.dt.float32, isOutput=True)

    with TileContext(nc) as tc:
        with (
            tc.tile_pool(name="const", bufs=1) as cpool,
            tc.tile_pool(name="xin", bufs=3) as xpool,
            tc.tile_pool(name="exp", bufs=3) as epool,
            tc.tile_pool(name="ln", bufs=2) as lpool,
            tc.tile_pool(name="acc", bufs=1) as apool,
            tc.tile_pool(name="ps", bufs=3, space="PSUM") as ppool,
        ):
            # selection matrix: col 0 sums the 100 class rows (norms),
            # col 1 picks row 100 (own-class density)
            sel = cpool.tile([P, 2], IN_DT)
            nc.sync.dma_start(sel[:, :], sel_in[:, :])

            # per-partition biases: col 0 = B0 (B' exp), col 1 = 0.0
            b0t = cpool.tile([P, 2], mybir.dt.float32)
            nc.vector.memset(b0t[:, 0:1], B0)
            nc.vector.memset(b0t[:, 1:2], 0.0)

            # tiny warm-up Exp: triggers the (single, patched) ACT table
            # load while the input DMA is in flight
            warm = cpool.tile([2, 2], mybir.dt.float32)
            nc.scalar.activation(
                warm[:, 0:1], b0t[0:2, 1:2], mybir.ActivationFunctionType.Exp,
                bias=b0t[0:2, 1:2],
            )

            acc = apool.tile([66, NGRP], mybir.dt.float32)

            xts = []
            for k, w in enumerate(CHUNKS):
                xt = xpool.tile([P, w], IN_DT, name=f"xt{k}", tag=f"xt{k}")
                nc.gpsimd.dma_start(xt[:, :], xs[k][:, :])
                xts.append(xt[:, :])

            # global bank g (0..7) -> psum group g//3, partition offset 32*(g%3)
            ps_tiles = {}
            g_abs = 0
            for k, w in enumerate(CHUNKS):
                bt = epool.tile([P, w], IN_DT, name=f"bt{k}", tag="bt")
                nc.scalar.activation(
                    bt[:, :], xts[k], mybir.ActivationFunctionType.Exp,
                    scale=DLT, bias=b0t[:, 0:1],
                )
                at = epool.tile([P, w], IN_DT, name=f"at{k}", tag="at")
                nc.scalar.activation(
                    at[:, :], xts[k], mybir.ActivationFunctionType.Exp,
                    scale=R0, bias=b0t[:, 1:2],
                )
                t0 = epool.tile([P, w], IN_DT, name=f"t0_{k}", tag="t0")
                nc.vector.tensor_scalar_add(t0[:, :], bt[:, :], W1P)
                u1 = epool.tile([P, w], IN_DT, name=f"u1_{k}", tag="u1")
                nc.vector.tensor_mul(u1[:, :], t0[:, :], bt[:, :])
                t1 = epool.tile([P, w], IN_DT, name=f"t1_{k}", tag="t0")
                nc.vector.tensor_scalar_add(t1[:, :], u1[:, :], W0)
                st = epool.tile([P, w], IN_DT, name=f"st{k}", tag="st")
                nc.vector.tensor_mul(st[:, :], t1[:, :], at[:, :])

                for b in range(w // MM_N):
                    grp, pos = divmod(g_abs, 3)
                    if pos == 0:
                        ps_tiles[grp] = ppool.tile(
                            [66, MM_N], mybir.dt.float32, name=f"ps{grp}", tag="ps"
                        )
                    nc.tensor.matmul(
                        ps_tiles[grp][32 * pos : 32 * pos + 2, :],
                        sel[:, :],
                        st[:, b * MM_N : (b + 1) * MM_N],
                        start=True,
                        stop=True,
                    )
                    if g_abs in (2, 5, 7):
                        lt = lpool.tile(
                            [66, MM_N], mybir.dt.float32, name=f"lt{grp}", tag="lt"
                        )
                        nc.scalar.activation(
                            lt[:, :],
                            ps_tiles[grp][:, :],
                            mybir.ActivationFunctionType.Ln,
                            bias=b0t[0:66, 1:2],
                            accum_out=acc[:, grp : grp + 1],
                        )
                        nc.sync.dma_start(
                            out[:, grp : grp + 1], acc[:, grp : grp + 1]
                        )
                    g_abs += 1

    nc.finalize()
    return nc


def _get_nc():
    if "nc" not in _CACHE:
        _CACHE["nc"] = build_bass()
    return _CACHE["nc"]


def kernel(features: np.ndarray, labels: np.ndarray) -> np.ndarray:
    global LAST_RESULT
    features = np.asarray(features)
    labels = np.asarray(labels)

    # view-major flatten: [B, V, D] -> [V*B, D]
    feats = np.ascontiguousarray(features.transpose(1, 0, 2).reshape(N, D))
    labels_rep = np.tile(labels.astype(np.int64), V)
    alab = feats[np.arange(N), labels_rep]  # own-class coordinate per column

    sel_np = np.zeros((P, 2), dtype=np.float16)
    sel_np[:C, 0] = 1.0
    sel_np[C, 1] = 1.0

    bounds = np.cumsum([0] + CHUNKS)
    in_maps = []
    for i in range(N_CORES):
        sl = slice(i * NLOC, (i + 1) * NLOC)
        X = np.zeros((P, NLOC), dtype=np.float16)
        X[:C, :] = feats[sl, :C].T
        X[C, :] = alab[sl]
        m = {"sel": sel_np}
        for k in range(len(CHUNKS)):
            m[f"x{k}"] = np.ascontiguousarray(X[:, bounds[k] : bounds[k + 1]])
        in_maps.append(m)

    nc = _get_nc()
    res = run_bass_kernel_spmd(nc, in_maps, list(range(N_CORES)), trace=TRACE)
    LAST_RESULT = res

    # group g holds banks 3g..min(3g+2,7) at partition offsets 0/32/64
    total = np.float64(0.0)
    for i in range(N_CORES):
        o = res.results[i]["out"].astype(np.float64)
        for g in range(NGRP):
            nb = 3 if g < 2 else 2
            for pos in range(nb):
                total += o[32 * pos, g] - o[32 * pos + 1, g]
    return np.asarray(total, dtype=np.float64)


# revision 40
# speedup vs baseline: 1.0701x; 1.0701x over previous
"""AGD loss (angular-Gaussian density contrastive loss) on 8 TRN2 NeuronCores.

Math.  Per column j (n = V*B = 32768 view-major columns) and class c (C = 100)
the reference evaluates the 40-term Saw-series density s(y[c,j]),
    s(a) = sum_n c_n a^n,   c_n = 2^{n/2} Gamma((d+n)/2) / (Gamma(d/2) n!),
takes norms_j = sum_c s(y[c,j]) and the own-class s(y[label_j, j]), and sums
-(log s_lab - log norms).  The huge exp(log_Cd - 1/(2 sigma^2)) prefactor
cancels in the log-ratio, so the kernel works with s directly in 16/32-bit.

The key identity: c_n * n! are exactly the raw moments of a chi(d=128)
variable R, so s(a) = E_R[e^{R a}].  A 3-node equispaced-exponential fit
    s(a) ~= A * ((B' + W1P) * B' + W0),   B' = e^{DLT a + B0},  A = e^{R0 a}
(nodes R0, R0+DLT, R0+2DLT; leading weight folded into B0) reproduces s to
1.9e-5 max rel on |a| <= 0.6; the constants are then re-centered against the
exact fp16 rounding staircase of the device pipeline (bias+variance objective
on the enumerable fp16 grid, weighted by the unit-sphere coordinate density),
which drives the end-to-end loss error to ~2e-7.

Per core (data-parallel over columns, 4096 columns/core, shard = rows
0..99 = y classes, row 100 = host-gathered own-class value, 101..127 zero):
  - input fp16 [128, w] chunks (512/1536/1536/512 columns), SWDGE DMA
    (128 partitions => full 16-engine spray, ~330 GB/s)
  - ScalarE: 2 Exp passes per chunk (B', A) with fused scale+bias
  - VectorE: quadratic Horner in fp16: tensor_scalar add (4x mode) +
    tensor_tensor mult (2x mode), twice
  - TensorE: [128 -> 2] matmul per 512-column bank against a constant
    (ones | one-hot-row-100) selection matrix; banks of 3 land at PSUM
    partition offsets 0/32/64 of a shared tile
  - ScalarE: one Ln (FD=512) per 3-bank group with accum_out -> [66, 3]
    partial sums; per-group DMA out
  - host: loss = sum over (norm - lab) partial-sum pairs in float64
All activation biases are explicit SBUF tiles (no const-AP reads), which
allows skipping the init-time all-engine barrier; the Tile tail is trimmed
to the global drain (re-execution verified bit-identical).
"""

import numpy as np

import concourse.bass as bass
import concourse.bacc as bacc
import concourse.mybir as mybir
from concourse.tile import TileContext
from concourse.bass_utils import run_bass_kernel_spmd

N_CORES = 8
B = 16384
V = 2
D = 128
C = 100
N = V * B                 # 32768 columns
NLOC = N // N_CORES       # 4096 columns per core
P = 128                   # 100 class rows + 1 own-class row + 27 zero pad rows
MM_N = 512                # matmul moving free dim (one PSUM bank)
NGRP = 3                  # Ln groups: banks {0,1,2}, {3,4,5}, {6,7}

# Equispaced 3-node exponential fit of the chi(128) MGF on |a| <= 0.6, with
# the leading weight folded into the B exponent and all constants re-centered
# against the exact fp16 rounding staircase of this pipeline:
#   s(a) ~= A * ((B' + W1P) * B' + W0),  B' = e^{DLT a + B0},  A = e^{R0 a}
# (raw fit 1.9e-5 max rel; tuned end-to-end loss error ~6e-7)
R0 = 9.785
DLT = 1.3
B0 = -0.7141612172077164
W1P = 1.387914841360724
W0 = 0.08100894876678372

CHUNKS = [512, 1536, 1536, 512]   # columns per chunk (banks 1/3/3/1)

IN_DT = mybir.dt.float16

_CACHE = {}
LAST_RESULT = None  # BassKernelResults of the most recent run (for profiling)
TRACE = False


class _scoped_patches:
    """Scoped (build-time only) framework tweaks:
    - Tile end-of-kernel: keep only the global drain (it waits for all
      outstanding work incl. the output DMAs); skip the per-semaphore clear
      storm and the two all-engine barriers.  Re-execution stays correct
      (verified bit-identical across runs) since semaphore state is reset
      at NEFF (re)start.
    - Map both Exp and Ln onto the single natural_log_exp_and_others ACT
      table set (one ~2.7us table load instead of two).
    - Skip the Bass-init all-engine barrier; its only role here is ordering
      the const-AP memsets, which this kernel never reads (all activation
      biases are explicit tiles)."""

    def __enter__(self):
        from concourse import tile as tile_mod
        import concourse.hw_specs as hw_specs

        def drain_only(tc_self, tick_clock, wait_clock):
            drain_inst = tc_self.nc.sync.drain()
            wait_clock.add_sem_waits(
                drain_inst.ins,
                tile_mod.ScopedClock({None: tick_clock.global_clock}),
            )
            popped = tc_self.nc._tile_sem_poison_stack.pop()
            assert popped is tc_self._sem_poison

        orig_tables = hw_specs.get_activation_tables

        def patched_tables(module_arch):
            tabs = orig_tables(module_arch)
            exp_f = mybir.ActivationFunctionType.Exp
            ln_f = mybir.ActivationFunctionType.Ln
            out = {}
            for name, funcs in tabs.items():
                if name != "natural_log_exp_and_others" and (
                    exp_f in funcs or ln_f in funcs
                ):
                    funcs = funcs - {exp_f, ln_f}
                out[name] = funcs
            return out

        self._saved = (
            tile_mod.TileContext._drain_and_barrier,
            hw_specs.get_activation_tables,
            bacc.get_activation_tables,
            bass.Bass.all_engine_barrier,
        )
        self._mods = (tile_mod, hw_specs)
        tile_mod.TileContext._drain_and_barrier = drain_only
        hw_specs.get_activation_tables = patched_tables
        bacc.get_activation_tables = patched_tables
        bass.Bass.all_engine_barrier = lambda nc_self, **kw: None
        return self

    def __exit__(self, *exc):
        tile_mod, hw_specs = self._mods
        (
            tile_mod.TileContext._drain_and_barrier,
            hw_specs.get_activation_tables,
            bacc.get_activation_tables,
            bass.Bass.all_engine_barrier,
        ) = self._saved
        return False


def build_bass():
    with _scoped_patches():
        return _build_bass_inner()


def _build_bass_inner():
    nc = bacc.Bacc(None, target_bir_lowering=False)
    xs = [
        nc.declare_dram_parameter(f"x{k}", [P, w], IN_DT, isOutput=False)
        for k, w in enumerate(CHUNKS)
    ]
    sel_in = nc.declare_dram_parameter("sel", [P, 2], IN_DT, isOutput=False)
    out = nc.declare_dram_parameter("out", [66, NGRP], mybir.dt.float32, isOutput=True)

    with TileContext(nc) as tc:
        with (
            tc.tile_pool(name="const", bufs=1) as cpool,
            tc.tile_pool(name="xin", bufs=3) as xpool,
            tc.tile_pool(name="exp", bufs=3) as epool,
            tc.tile_pool(name="ln", bufs=2) as lpool,
            tc.tile_pool(name="acc", bufs=1) as apool,
            tc.tile_pool(name="ps", bufs=3, space="PSUM") as ppool,
        ):
            # selection matrix: col 0 sums the 100 class rows (norms),
            # col 1 picks row 100 (own-class density)
            sel = cpool.tile([P, 2], IN_DT)
            nc.sync.dma_start(sel[:, :], sel_in[:, :])

            # per-partition biases: col 0 = B0 (B' exp), col 1 = 0.0
            b0t = cpool.tile([P, 2], mybir.dt.float32)
            nc.vector.memset(b0t[:, 0:1], B0)
            nc.vector.memset(b0t[:, 1:2], 0.0)

            # tiny warm-up Exp: triggers the (single, patched) ACT table
            # load while the input DMA is in flight
            warm = cpool.tile([2, 2], mybir.dt.float32)
            nc.scalar.activation(
                warm[:, 0:1], b0t[0:2, 1:2], mybir.ActivationFunctionType.Exp,
                bias=b0t[0:2, 1:2],
            )

            acc = apool.tile([66, NGRP], mybir.dt.float32)

            xts = []
            for k, w in enumerate(CHUNKS):
                xt = xpool.tile([P, w], IN_DT, name=f"xt{k}", tag=f"xt{k}")
                nc.gpsimd.dma_start(xt[:, :], xs[k][:, :])
                xts.append(xt[:, :])

            # global bank g (0..7) -> psum group g//3, partition offset 32*(g%3)
            ps_tiles = {}
            g_abs = 0
            for k, w in enumerate(CHUNKS):
                bt = epool.tile([P, w], IN_DT, name=f"bt{k}", tag="bt")
                nc.scalar.activation(
                    bt[:, :], xts[k], mybir.ActivationFunctionType.Exp,
                    scale=DLT, bias=b0t[:, 0:1],
                )
                at = epool.tile([P, w], IN_DT, name=f"at{k}", tag="at")
                nc.scalar.activation(
                    at[:, :], xts[k], mybir.ActivationFunctionType.Exp,
                    scale=R0, bias=b0t[:, 1:2],
                )
                t0 = epool.tile([P, w], IN_DT, name=f"t0_{k}", tag="t0")
                nc.vector.tensor_scalar_add(t0[:, :], bt[:, :], W1P)
                u1 = epool.tile([P, w], IN_DT, name=f"u1_{k}", tag="u1")
                nc.vector.tensor_mul(u1[:, :], t0[:, :], bt[:, :])
                t1 = epool.tile([P, w], IN_DT, name=f"t1_{k}", tag="t0")
                nc.vector.tensor_scalar_add(t1[:, :], u1[:, :], W0)
                st = epool.tile([P, w], IN_DT, name=f"st{k}", tag="st")
                nc.vector.tensor_mul(st[:, :], t1[:, :], at[:, :])

                for b in range(w // MM_N):
                    grp, pos = divmod(g_abs, 3)
                    if pos == 0:
                        ps_tiles[grp] = ppool.tile(
                            [66, MM_N], mybir.dt.float32, name=f"ps{grp}", tag="ps"
                        )
                    nc.tensor.matmul(
                        ps_tiles[grp][32 * pos : 32 * pos + 2, :],
                        sel[:, :],
                        st[:, b * MM_N : (b + 1) * MM_N],
                        start=True,
                        stop=True,
                    )
                    if g_abs in (2, 5, 7):
                        lt = lpool.tile(
                            [66, MM_N], mybir.dt.float32, name=f"lt{grp}", tag="lt"
                        )
                        nc.scalar.activation(
                            lt[:, :],
                            ps_tiles[grp][:, :],
                            mybir.ActivationFunctionType.Ln,
                            bias=b0t[0:66, 1:2],
                            accum_out=acc[:, grp : grp + 1],
                        )
                        nc.sync.dma_start(
                            out[:, grp : grp + 1], acc[:, grp : grp + 1]
                        )
                    g_abs += 1

    nc.finalize()
    return nc


def _get_nc():
    if "nc" not in _CACHE:
        _CACHE["nc"] = build_bass()
    return _CACHE["nc"]


def kernel(features: np.ndarray, labels: np.ndarray) -> np.ndarray:
    global LAST_RESULT
    features = np.asarray(features)
    labels = np.asarray(labels)

    # view-major flatten: [B, V, D] -> [V*B, D]
    feats = np.ascontiguousarray(features.transpose(1, 0, 2).reshape(N, D))
    labels_rep = np.tile(labels.astype(np.int64), V)
    alab = feats[np.arange(N), labels_rep]  # own-class coordinate per column

    sel_np = np.zeros((P, 2), dtype=np.float16)
    sel_np[:C, 0] = 1.0
    sel_np[C, 1] = 1.0

    bounds = np.cumsum([0] + CHUNKS)
    in_maps = []
    for i in range(N_CORES):
        sl = slice(i * NLOC, (i + 1) * NLOC)
        X = np.zeros((P, NLOC), dtype=np.float16)
        X[:C, :] = feats[sl, :C].T
        X[C, :] = alab[sl]
        m = {"sel": sel_np}
        for k in range(len(CHUNKS)):
            m[f"x{k}"] = np.ascontiguousarray(X[:, bounds[k] : bounds[k + 1]])
        in_maps.append(m)

    nc = _get_nc()
    res = run_bass_kernel_spmd(nc, in_maps, list(range(N_CORES)), trace=TRACE)
    LAST_RESULT = res

    # group g holds banks 3g..min(3g+2,7) at partition offsets 0/32/64
    total = np.float64(0.0)
    for i in range(N_CORES):
        o = res.results[i]["out"].astype(np.float64)
        for g in range(NGRP):
            nb = 3 if g < 2 else 2
            for pos in range(nb):
                total += o[32 * pos, g] - o[32 * pos + 1, g]
    return np.asarray(total, dtype=np.float64)
